# revision 36
# baseline (speedup 1.0000x reference)
"""Causal single-head attention (B=8, S=E=1024) for 8 Trainium2 cores.

Strategy: data-parallel over batch — core b handles batch element b.
All matmul operands are bf16 (half the DMA/SBUF traffic, cheap PE
transposes, 2x DVE). PSUM accumulation stays fp32; measured end-to-end
rel err ~6e-3 vs fp32.

Phase order:
  1. kT[d,s] = WkT.T @ xT (+bk)   boot chunk eo-outer over 8 PSUM banks
  2. qT[d,s] = (Wq/32)T.T @ xT (+bq/32)  (1/32 folded into host weights)
  3. scores_i (descending i) = qT_i.T @ kT in 512-col chunks -> diag
     mask -> p_i = exp(scores) in bf16 with fused per-chunk row-sums
     (no max subtraction: |scores| <= ~8, fp32-safe), then V blocks 0/1
  4. V chunk 0 (blocks 2-7); V chunk 1 fused with AV_i per s-block, so
     each out tile DMAs out across the phase instead of at the end.

PSUM discipline (the thing that actually matters): every [128,512] f32
accumulation (boot, kT c1, qT, scores chunks, V blocks 0/1) allocates
from ONE 8-slot pool ring with a single tag, so bank reuse follows
eviction order FIFO with ~8 groups of slack — phase-boundary "fresh
pool" allocations otherwise inherit banks whose previous eviction piles
up at the old phase's end (start=True also clears has_written for the
WHOLE bank, so split accumulations may only carry it on the first
write). The V/AV phase gets a second 6-slot ring + 2 banks of
transpose staging; V blocks 0/1 ride the first ring so the handoff
hides under their matmuls.

DVE is strict FIFO: anything eligible late (exp-gated reciprocals)
head-of-line-blocks masks and stalls the scores bank rotation, so
reciprocals/lsum-merges all issue after the scores loop.

DMA: all weights load up-front into resident SBUF tiles (three rings,
issue order tracks consumption; per-queue throughput ~80 GB/s early).
Biases come host-packed [128, do] (a (o p)->p o gather DMA costs ~1us
issue + 4-byte packets). Output rotates over three rings during the
fused V/AV phase; the final tile is split four ways.
"""

import os
import sys
from contextlib import ExitStack

for _p in ("/opt/trn_rl_repo", "/root/.axon_site/_ro/trn_rl_repo"):
    if os.path.isdir(_p) and _p not in sys.path:
        sys.path.insert(0, _p)

import numpy as np
import ml_dtypes

import concourse.bass as bass
import concourse.mybir as mybir
import concourse.tile as tile
from concourse import bacc
from concourse.bass_utils import run_bass_kernel_spmd

P = 128
S = 1024
E = 1024
D = 1024
B = 8
SO = S // P
EO = E // P
DO = D // P
CH = 512
NCH = D // CH
SCALE = 1.0 / np.sqrt(float(E))  # 1/32
MASK_VAL = -1e9

F32 = mybir.dt.float32
BF16 = mybir.dt.bfloat16


def build_program():
    nc = bacc.Bacc(
        "TRN2", target_bir_lowering=False, debug=False, enable_asserts=True
    )

    xT = nc.dram_tensor("xT", [E, S], BF16, kind="ExternalInput").ap()
    wqT = nc.dram_tensor("wqT", [E, D], BF16, kind="ExternalInput").ap()  # *1/32
    wkT = nc.dram_tensor("wkT", [E, D], BF16, kind="ExternalInput").ap()
    wvT = nc.dram_tensor("wvT", [E, D], BF16, kind="ExternalInput").ap()
    bqs = nc.dram_tensor("bqs", [P, DO], F32, kind="ExternalInput").ap()  # bq/32
    bk = nc.dram_tensor("bk", [P, DO], F32, kind="ExternalInput").ap()
    bv = nc.dram_tensor("bv", [D], F32, kind="ExternalInput").ap()
    out = nc.dram_tensor("out", [S, D], BF16, kind="ExternalOutput").ap()

    with tile.TileContext(nc) as tc, ExitStack() as ctx:
        consts = ctx.enter_context(tc.tile_pool(name="consts", bufs=1))
        bigs = ctx.enter_context(tc.tile_pool(name="bigs", bufs=1))
        wres = ctx.enter_context(tc.tile_pool(name="wres", bufs=1))
        small = ctx.enter_context(tc.tile_pool(name="small", bufs=32))

        # resident tensors (all bf16)
        x_sb = bigs.tile([P, EO, S], BF16)
        kT_sb = bigs.tile([P, DO, S], BF16)
        qT_sb = bigs.tile([P, DO, S], BF16)
        v_sb = bigs.tile([P, SO, D], BF16)
        p_all = bigs.tile([P, SO, S], BF16)  # exp(scores) for every q-tile

        wq_r = wqT.rearrange("(eo p) o -> p eo o", p=P)
        wk_r = wkT.rearrange("(eo p) o -> p eo o", p=P)
        wv_r = wvT.rearrange("(eo p) o -> p eo o", p=P)

        # identity/cmask: memsets on the idle DVE; affine_select is
        # gpsimd-only, identity's runs before the gpsimd DMA issues and
        # cmask's after (it isn't consumed until the scores phase)
        identity = consts.tile([P, P], BF16)
        cmask = consts.tile([P, P], F32)
        nc.vector.memset(identity, 0.0)
        nc.vector.memset(cmask, 0.0)
        nc.gpsimd.affine_select(
            out=identity,
            in_=identity,
            compare_op=mybir.AluOpType.not_equal,
            fill=1.0,
            base=0,
            pattern=[[-1, P]],
            channel_multiplier=1,
        )

        # ---- startup: every input DMA issues up front, ordered so arrival
        # tracks consumption; all weights are resident so no phase waits on
        # a weight arrival.
        wk0_pool = ctx.enter_context(tc.tile_pool(name="wk0_pool", bufs=1))
        wk0 = wk0_pool.tile([P, EO, CH], BF16, name="wk0")
        wk1 = wk0_pool.tile([P, EO, CH], BF16, name="wk1")
        x_r = xT.rearrange("(eo p) s -> p eo s", p=P)
        nc.sync.dma_start(wk0[:, 0, :], wk_r[:, 0, 0:CH])
        nc.scalar.dma_start(x_sb[:, 0, 0:CH], x_r[:, 0, 0:CH])
        nc.gpsimd.dma_start(x_sb[:, 0, CH:S], x_r[:, 0, CH:S])
        nc.scalar.dma_start(x_sb[:, 1, :], x_r[:, 1, :])
        nc.gpsimd.dma_start(wk0[:, 1, :], wk_r[:, 1, 0:CH])
        nc.scalar.dma_start(x_sb[:, 2, :], x_r[:, 2, :])
        nc.scalar.dma_start(x_sb[:, 3, :], x_r[:, 3, :])
        nc.gpsimd.dma_start(wk0[:, 2:4, :], wk_r[:, 2:4, 0:CH])
        nc.scalar.dma_start(x_sb[:, 4, :], x_r[:, 4, :])
        nc.sync.dma_start(x_sb[:, 5, :], x_r[:, 5, :])
        nc.gpsimd.dma_start(wk0[:, 4:6, :], wk_r[:, 4:6, 0:CH])
        nc.scalar.dma_start(x_sb[:, 6, :], x_r[:, 6, :])
        nc.gpsimd.dma_start(wk0[:, 6:8, :], wk_r[:, 6:8, 0:CH])
        nc.sync.dma_start(x_sb[:, 7, :], x_r[:, 7, :])
        # packed biases (one clean [P, DO] DMA each; bk gates boot evicts)
        bk_t = consts.tile([P, DO], F32)
        nc.gpsimd.dma_start(bk_t, bk)
        bq_t = consts.tile([P, DO], F32)
        nc.gpsimd.dma_start(bq_t, bqs)
        # wk1 halves next (consumed right after the boot)
        nc.sync.dma_start(wk1[:, 0:4, :], wk_r[:, 0:4, CH : 2 * CH])
        nc.scalar.dma_start(wk1[:, 4:8, :], wk_r[:, 4:8, CH : 2 * CH])
        # remaining weights: resident tiles
        wq0 = wres.tile([P, EO, CH], BF16, name="wq0")
        wq1 = wres.tile([P, EO, CH], BF16, name="wq1")
        wv0 = wres.tile([P, EO, CH], BF16, name="wv0")
        wv1 = wres.tile([P, EO, CH], BF16, name="wv1")
        for wt, w_r, c, second in (
            (wq0, wq_r, 0, nc.gpsimd),
            (wq1, wq_r, 1, nc.gpsimd),
            (wv0, wv_r, 0, nc.scalar),
            (wv1, wv_r, 1, nc.scalar),
        ):
            nc.sync.dma_start(wt[:, 0:4, :], w_r[:, 0:4, c * CH : (c + 1) * CH])
            second.dma_start(wt[:, 4:8, :], w_r[:, 4:8, c * CH : (c + 1) * CH])
        # bv broadcast across partitions (V evictions only, late; its issue
        # costs ~4.6us of scalar-queue time, after everything that matters)
        bv_b = consts.tile([P, D], F32)
        nc.scalar.dma_start(bv_b, bv[None, :].broadcast_to([P, D]))
        # causal mask fill (consumed at the scores phase)
        nc.gpsimd.affine_select(
            out=cmask,
            in_=cmask,
            compare_op=mybir.AluOpType.is_ge,
            fill=MASK_VAL,
            base=0,
            pattern=[[-1, P]],
            channel_multiplier=1,
        )

        def project_chunk(ring, wt, c, dst, bias_t):
            # dst[d_part, do, s] (+bias per-partition) for d-chunk c;
            # evictions alternate Act/DVE
            for dj in range(CH // P):
                do = c * (CH // P) + dj
                for ch in range(S // CH):
                    ps = ring.tile([P, CH], F32, tag="acc", name="ps")
                    for eo in range(EO):
                        nc.tensor.matmul(
                            ps,
                            lhsT=wt[:, eo, dj * P : (dj + 1) * P],
                            rhs=x_sb[:, eo, ch * CH : (ch + 1) * CH],
                            start=(eo == 0),
                            stop=(eo == EO - 1),
                        )
                    if (dj + ch) % 2 == 0:
                        nc.scalar.activation(
                            dst[:, do, ch * CH : (ch + 1) * CH],
                            ps,
                            mybir.ActivationFunctionType.Identity,
                            bias=bias_t[:, do : do + 1],
                            scale=1.0,
                        )
                    else:
                        nc.vector.tensor_scalar(
                            dst[:, do, ch * CH : (ch + 1) * CH],
                            ps,
                            bias_t[:, do : do + 1],
                            None,
                            mybir.AluOpType.add,
                        )

        lsums = {}
        rinvs = {}

        # ---- ring 1: boot (kT c0) + kT c1 + qT + scores + V blocks 0/1,
        # all through one 8-slot [128,512] ring: bank reuse is FIFO in
        # eviction order ----
        with tc.tile_pool(name="ring1", bufs=8, space="PSUM") as ring1:
            # ch-major: the first 4 matmuls of each eo round share one
            # 512-col x block, so the boot starts on the earliest arrival
            groups = [(dj, ch) for ch in range(S // CH) for dj in range(CH // P)]
            boot_tiles = [
                ring1.tile([P, CH], F32, tag="acc", name=f"bps_{g}")
                for g in range(len(groups))
            ]
            # eo-outer over all 8 banks: consumption (~3.4us/eo cold)
            # tracks the early x arrival rate
            for eo in range(EO):
                for g, (dj, ch) in enumerate(groups):
                    nc.tensor.matmul(
                        boot_tiles[g],
                        lhsT=wk0[:, eo, dj * P : (dj + 1) * P],
                        rhs=x_sb[:, eo, ch * CH : (ch + 1) * CH],
                        start=(eo == 0),
                        stop=(eo == EO - 1),
                    )
            for g, (dj, ch) in enumerate(groups):
                if g % 2 == 0:
                    nc.scalar.activation(
                        kT_sb[:, dj, ch * CH : (ch + 1) * CH],
                        boot_tiles[g],
                        mybir.ActivationFunctionType.Identity,
                        bias=bk_t[:, dj : dj + 1],
                        scale=1.0,
                    )
                else:
                    nc.vector.tensor_scalar(
                        kT_sb[:, dj, ch * CH : (ch + 1) * CH],
                        boot_tiles[g],
                        bk_t[:, dj : dj + 1],
                        None,
                        mybir.AluOpType.add,
                    )

            project_chunk(ring1, wk1, 1, kT_sb, bk_t)
            project_chunk(ring1, wq0, 0, qT_sb, bq_t)
            project_chunk(ring1, wq1, 1, qT_sb, bq_t)

            # ---- scores, descending i, in 512-col chunks on the same ring.
            # Per chunk: 8 accumulating matmuls over do, diag mask (DVE) on
            # the last chunk, exp on Act with fused per-chunk row-sum.
            for i in reversed(range(SO)):
                kw = (i + 1) * P
                nch = (kw + CH - 1) // CH
                for ch in range(nch):
                    w = min(CH, kw - ch * CH)
                    ps_c = ring1.tile([P, CH], F32, tag="acc", name="ps_sc")
                    for do in range(DO):
                        nc.tensor.matmul(
                            ps_c[:, 0:w],
                            lhsT=qT_sb[:, do, i * P : (i + 1) * P],
                            rhs=kT_sb[:, do, ch * CH : ch * CH + w],
                            start=(do == 0),
                            stop=(do == DO - 1),
                        )
                    if ch == nch - 1:
                        # additive causal mask on the diagonal block
                        nc.vector.tensor_tensor(
                            ps_c[:, w - P : w],
                            ps_c[:, w - P : w],
                            cmask,
                            mybir.AluOpType.add,
                        )
                    nc.scalar.activation(
                        p_all[:, i, ch * CH : ch * CH + w],
                        ps_c[:, 0:w],
                        mybir.ActivationFunctionType.Exp,
                        bias=0.0,
                        scale=1.0,
                    )

            # V c0 blocks 0/1 ride the tail of ring1 (slots freed many
            # groups ago) so the ring1->ring2 handoff hides under them
            for so in (0, 1):
                ps = ring1.tile([P, CH], F32, tag="acc", name="ps_v")
                for eo in range(EO):
                    nc.tensor.matmul(
                        ps,
                        lhsT=x_sb[:, eo, so * P : (so + 1) * P],
                        rhs=wv0[:, eo, :],
                        start=(eo == 0),
                        stop=(eo == EO - 1),
                    )
                nc.vector.tensor_tensor(
                    v_sb[:, so, 0:CH],
                    ps,
                    bv_b[:, 0:CH],
                    mybir.AluOpType.add,
                )

        # row-sums + reciprocals on DVE, emitted after the scores loop:
        # Act is saturated by the exps (accum_out reads would add ~3.4us
        # and push the last exp far past the phase end), and DVE is strict
        # FIFO so exp-gated work interleaved with masks would block the
        # scores bank rotation. Consumed only by the AV evictions later.
        for i in range(SO):
            kw = (i + 1) * P
            lsum = small.tile([P, 1], F32, tag="lsum", name=f"ls{i}")
            nc.vector.tensor_reduce(
                lsum, p_all[:, i, 0:kw], mybir.AxisListType.X,
                mybir.AluOpType.add,
            )
            rinv = small.tile([P, 1], F32, tag="rinv", name=f"rinv{i}")
            nc.vector.reciprocal(rinv, lsum)
            rinvs[i] = rinv

        # ---- ring 2 (rest of V + AV accumulations) + transpose staging ----
        ring2 = ctx.enter_context(tc.tile_pool(name="ring2", bufs=6, space="PSUM"))
        tr_ps = ctx.enter_context(tc.tile_pool(name="tr_ps", bufs=2, space="PSUM"))
        pt_pool = ctx.enter_context(tc.tile_pool(name="pt_pool", bufs=5))
        out_pool = ctx.enter_context(tc.tile_pool(name="out_pool", bufs=4))

        def emit_transposes(i):
            # pT_i = p_i blocks transposed via the PE; copies drain on DVE
            pT = pt_pool.tile([P, S], BF16, tag="pT", name=f"pT{i}")
            for j in range(i + 1):
                ps_t = tr_ps.tile([P, P], BF16, tag="tr", name="ps_t")
                nc.tensor.transpose(
                    ps_t, p_all[:, i, j * P : (j + 1) * P], identity
                )
                nc.vector.tensor_copy(pT[:, j * P : (j + 1) * P], ps_t)
            return pT

        def emit_av(i):
            # out_i = (pT_i.T @ v) * rinv_i, evicted per 512-chunk; out DMAs
            # rotate over three rings so writeback streams during the phase.
            nk = i + 1
            rinv = rinvs[i]
            pT = pts.pop(i)
            out_sb = out_pool.tile([P, D], BF16, tag="out", name="out_sb")
            last = i == SO - 1
            rings = (nc.gpsimd, nc.sync, nc.scalar)
            for c2 in range(NCH):
                ps_o = ring2.tile([P, CH], F32, tag="av", name="ps_o")
                for j in range(nk):
                    nc.tensor.matmul(
                        ps_o,
                        lhsT=pT[:, j * P : (j + 1) * P],
                        rhs=v_sb[:, j, c2 * CH : (c2 + 1) * CH],
                        start=(j == 0),
                        stop=(j == nk - 1),
                    )
                base = c2 * CH
                if last and c2 == NCH - 1:
                    # final tile: eviction split DVE+Act, output quartered
                    # across the rings to shorten the tail drain
                    half = CH // 2
                    qt = CH // 4
                    nc.vector.tensor_scalar_mul(
                        out_sb[:, base : base + half], ps_o[:, 0:half], rinv
                    )
                    nc.scalar.activation(
                        out_sb[:, base + half : base + CH],
                        ps_o[:, half:CH],
                        mybir.ActivationFunctionType.Identity,
                        bias=0.0,
                        scale=rinv,
                    )
                    for qi, ring in enumerate(
                        (nc.gpsimd, nc.sync, nc.scalar, nc.sync)
                    ):
                        ring.dma_start(
                            out[
                                i * P : (i + 1) * P,
                                base + qi * qt : base + (qi + 1) * qt,
                            ],
                            out_sb[:, base + qi * qt : base + (qi + 1) * qt],
                        )
                else:
                    # alternate evictions DVE/Act so neither queue stalls
                    # the next tile's pT copies
                    if c2 % 2 == 0:
                        nc.vector.tensor_scalar_mul(
                            out_sb[:, base : base + CH], ps_o, rinv
                        )
                    else:
                        nc.scalar.activation(
                            out_sb[:, base : base + CH],
                            ps_o,
                            mybir.ActivationFunctionType.Identity,
                            bias=0.0,
                            scale=rinv,
                        )
                    ring = rings[(i * NCH + c2) % 3]
                    ring.dma_start(
                        out[i * P : (i + 1) * P, base : base + CH],
                        out_sb[:, base : base + CH],
                    )

        # ---- V chunk 0 (blocks 2-7), then V chunk 1 fused with AV ----
        pts = {}
        for so in range(2, SO):
            ps = ring2.tile([P, CH], F32, tag="av", name="ps_v")
            for eo in range(EO):
                nc.tensor.matmul(
                    ps,
                    lhsT=x_sb[:, eo, so * P : (so + 1) * P],
                    rhs=wv0[:, eo, :],
                    start=(eo == 0),
                    stop=(eo == EO - 1),
                )
            nc.vector.tensor_tensor(
                v_sb[:, so, 0:CH],
                ps,
                bv_b[:, 0:CH],
                mybir.AluOpType.add,
            )
            if so >= SO - 4:
                # pT_0..pT_3 prefire under the tail of the V c0 matmuls
                pts[so - (SO - 4)] = emit_transposes(so - (SO - 4))
        for so in range(SO):
            ps = ring2.tile([P, CH], F32, tag="av", name="ps_v")
            for eo in range(EO):
                nc.tensor.matmul(
                    ps,
                    lhsT=x_sb[:, eo, so * P : (so + 1) * P],
                    rhs=wv1[:, eo, :],
                    start=(eo == 0),
                    stop=(eo == EO - 1),
                )
            nc.vector.tensor_tensor(
                v_sb[:, so, CH : 2 * CH],
                ps,
                bv_b[:, CH : 2 * CH],
                mybir.AluOpType.add,
            )
            if so + 4 < SO:
                pts[so + 4] = emit_transposes(so + 4)
            emit_av(so)

    nc.compile()
    return nc


_NC_CACHE = None


def get_program():
    global _NC_CACHE
    if _NC_CACHE is None:
        _NC_CACHE = build_program()
    return _NC_CACHE


def make_in_maps(x, Wq, bq, Wk, bk, Wv, bv):
    x = np.asarray(x, dtype=np.float32)
    wqT = np.ascontiguousarray(
        (np.asarray(Wq, dtype=np.float32).T * np.float32(SCALE)).astype(
            ml_dtypes.bfloat16
        )
    )
    wkT = np.ascontiguousarray(np.asarray(Wk, dtype=np.float32).T.astype(ml_dtypes.bfloat16))
    wvT = np.ascontiguousarray(np.asarray(Wv, dtype=np.float32).T.astype(ml_dtypes.bfloat16))
    # biases packed to [P, DO]: element (p, o) = b[o*P + p]
    bqs = np.ascontiguousarray(
        (np.asarray(bq, dtype=np.float32) * np.float32(SCALE)).reshape(DO, P).T
    )
    bk = np.ascontiguousarray(np.asarray(bk, dtype=np.float32).reshape(DO, P).T)
    bv = np.asarray(bv, dtype=np.float32)
    in_maps = []
    for b in range(B):
        in_maps.append(
            {
                "xT": np.ascontiguousarray(x[b].T.astype(ml_dtypes.bfloat16)),
                "wqT": wqT,
                "wkT": wkT,
                "wvT": wvT,
                "bqs": bqs,
                "bk": bk,
                "bv": bv,
            }
        )
    return in_maps


def run_on_hw(in_maps, trace=False, **kwargs):
    nc = get_program()
    return run_bass_kernel_spmd(
        nc, in_maps, core_ids=list(range(B)), trace=trace, **kwargs
    )


def kernel(x, Wq, bq, Wk, bk, Wv, bv):
    in_maps = make_in_maps(x, Wq, bq, Wk, bk, Wv, bv)
    res = run_on_hw(in_maps)
    return np.stack(
        [res.results[b]["out"].astype(np.float32) for b in range(B)], axis=0
    )


# revision 37
# speedup vs baseline: 1.0144x; 1.0144x over previous
"""Causal single-head attention (B=8, S=E=1024) for 8 Trainium2 cores.

Strategy: data-parallel over batch — core b handles batch element b.
All matmul operands are bf16 (half the DMA/SBUF traffic, cheap PE
transposes, 2x DVE). PSUM accumulation stays fp32; measured end-to-end
rel err ~6e-3 vs fp32.

Phase order:
  1. kT[d,s] = WkT.T @ xT (+bk)   boot chunk eo-outer over 8 PSUM banks
  2. qT[d,s] = (Wq/32)T.T @ xT (+bq/32)  (1/32 folded into host weights)
  3. scores_i (descending i) = qT_i.T @ kT in 512-col chunks -> diag
     mask -> p_i = exp(scores) in bf16 with fused per-chunk row-sums
     (no max subtraction: |scores| <= ~8, fp32-safe), then V blocks 0/1
  4. V chunk 0 (blocks 2-7); V chunk 1 fused with AV_i per s-block, so
     each out tile DMAs out across the phase instead of at the end.

PSUM discipline (the thing that actually matters): every [128,512] f32
accumulation (boot, kT c1, qT, scores chunks, V blocks 0/1) allocates
from ONE 8-slot pool ring with a single tag, so bank reuse follows
eviction order FIFO with ~8 groups of slack — phase-boundary "fresh
pool" allocations otherwise inherit banks whose previous eviction piles
up at the old phase's end (start=True also clears has_written for the
WHOLE bank, so split accumulations may only carry it on the first
write). The V/AV phase gets a second 6-slot ring + 2 banks of
transpose staging; V blocks 0/1 ride the first ring so the handoff
hides under their matmuls.

DVE is strict FIFO: anything eligible late (exp-gated reciprocals)
head-of-line-blocks masks and stalls the scores bank rotation, so
reciprocals/lsum-merges all issue after the scores loop.

DMA: all weights load up-front into resident SBUF tiles (three rings,
issue order tracks consumption; per-queue throughput ~80 GB/s early).
Biases come host-packed [128, do] (a (o p)->p o gather DMA costs ~1us
issue + 4-byte packets). Output rotates over three rings during the
fused V/AV phase; the final tile is split four ways.
"""

import os
import sys
from contextlib import ExitStack

for _p in ("/opt/trn_rl_repo", "/root/.axon_site/_ro/trn_rl_repo"):
    if os.path.isdir(_p) and _p not in sys.path:
        sys.path.insert(0, _p)

import numpy as np
import ml_dtypes

import concourse.bass as bass
import concourse.mybir as mybir
import concourse.tile as tile
from concourse import bacc
from concourse.bass_utils import run_bass_kernel_spmd

P = 128
S = 1024
E = 1024
D = 1024
B = 8
SO = S // P
EO = E // P
DO = D // P
CH = 512
NCH = D // CH
SCALE = 1.0 / np.sqrt(float(E))  # 1/32
MASK_VAL = -1e9

F32 = mybir.dt.float32
BF16 = mybir.dt.bfloat16


def build_program():
    nc = bacc.Bacc(
        "TRN2", target_bir_lowering=False, debug=False, enable_asserts=True
    )

    xT = nc.dram_tensor("xT", [E, S], BF16, kind="ExternalInput").ap()
    wqT = nc.dram_tensor("wqT", [E, D], BF16, kind="ExternalInput").ap()  # *1/32
    wkT = nc.dram_tensor("wkT", [E, D], BF16, kind="ExternalInput").ap()
    wvT = nc.dram_tensor("wvT", [E, D], BF16, kind="ExternalInput").ap()
    bqs = nc.dram_tensor("bqs", [P, DO], F32, kind="ExternalInput").ap()  # bq/32
    bk = nc.dram_tensor("bk", [P, DO], F32, kind="ExternalInput").ap()
    bv = nc.dram_tensor("bv", [D], F32, kind="ExternalInput").ap()
    out = nc.dram_tensor("out", [S, D], BF16, kind="ExternalOutput").ap()

    with tile.TileContext(nc) as tc, ExitStack() as ctx:
        consts = ctx.enter_context(tc.tile_pool(name="consts", bufs=1))
        bigs = ctx.enter_context(tc.tile_pool(name="bigs", bufs=1))
        wres = ctx.enter_context(tc.tile_pool(name="wres", bufs=1))
        small = ctx.enter_context(tc.tile_pool(name="small", bufs=32))

        # resident tensors (all bf16)
        x_sb = bigs.tile([P, EO, S], BF16)
        kT_sb = bigs.tile([P, DO, S], BF16)
        qT_sb = bigs.tile([P, DO, S], BF16)
        v_sb = bigs.tile([P, SO, D], BF16)
        p_all = bigs.tile([P, SO, S], BF16)  # exp(scores) for every q-tile

        wq_r = wqT.rearrange("(eo p) o -> p eo o", p=P)
        wk_r = wkT.rearrange("(eo p) o -> p eo o", p=P)
        wv_r = wvT.rearrange("(eo p) o -> p eo o", p=P)

        # identity/cmask: memsets on the idle DVE; affine_select is
        # gpsimd-only, identity's runs before the gpsimd DMA issues and
        # cmask's after (it isn't consumed until the scores phase)
        identity = consts.tile([P, P], BF16)
        cmask = consts.tile([P, P], F32)
        nc.vector.memset(identity, 0.0)
        nc.vector.memset(cmask, 0.0)
        nc.gpsimd.affine_select(
            out=identity,
            in_=identity,
            compare_op=mybir.AluOpType.not_equal,
            fill=1.0,
            base=0,
            pattern=[[-1, P]],
            channel_multiplier=1,
        )

        # ---- startup: every input DMA issues up front, ordered so arrival
        # tracks consumption; all weights are resident so no phase waits on
        # a weight arrival.
        wk0_pool = ctx.enter_context(tc.tile_pool(name="wk0_pool", bufs=1))
        wk0 = wk0_pool.tile([P, EO, CH], BF16, name="wk0")
        wk1 = wk0_pool.tile([P, EO, CH], BF16, name="wk1")
        x_r = xT.rearrange("(eo p) s -> p eo s", p=P)
        nc.sync.dma_start(wk0[:, 0, :], wk_r[:, 0, 0:CH])
        nc.scalar.dma_start(x_sb[:, 0, 0:CH], x_r[:, 0, 0:CH])
        nc.gpsimd.dma_start(x_sb[:, 0, CH:S], x_r[:, 0, CH:S])
        nc.scalar.dma_start(x_sb[:, 1, :], x_r[:, 1, :])
        nc.gpsimd.dma_start(wk0[:, 1, :], wk_r[:, 1, 0:CH])
        nc.sync.dma_start(x_sb[:, 2, :], x_r[:, 2, :])
        nc.scalar.dma_start(x_sb[:, 3, :], x_r[:, 3, :])
        nc.gpsimd.dma_start(wk0[:, 2:4, :], wk_r[:, 2:4, 0:CH])
        nc.scalar.dma_start(x_sb[:, 4, :], x_r[:, 4, :])
        nc.sync.dma_start(x_sb[:, 5, :], x_r[:, 5, :])
        nc.gpsimd.dma_start(wk0[:, 4:6, :], wk_r[:, 4:6, 0:CH])
        nc.scalar.dma_start(x_sb[:, 6, :], x_r[:, 6, :])
        nc.gpsimd.dma_start(wk0[:, 6:8, :], wk_r[:, 6:8, 0:CH])
        nc.sync.dma_start(x_sb[:, 7, :], x_r[:, 7, :])
        # packed biases (one clean [P, DO] DMA each; bk gates boot evicts)
        bk_t = consts.tile([P, DO], F32)
        nc.gpsimd.dma_start(bk_t, bk)
        bq_t = consts.tile([P, DO], F32)
        nc.gpsimd.dma_start(bq_t, bqs)
        # wk1 halves next (consumed right after the boot)
        nc.sync.dma_start(wk1[:, 0:4, :], wk_r[:, 0:4, CH : 2 * CH])
        nc.scalar.dma_start(wk1[:, 4:8, :], wk_r[:, 4:8, CH : 2 * CH])
        # remaining weights: resident tiles
        wq0 = wres.tile([P, EO, CH], BF16, name="wq0")
        wq1 = wres.tile([P, EO, CH], BF16, name="wq1")
        wv0 = wres.tile([P, EO, CH], BF16, name="wv0")
        wv1 = wres.tile([P, EO, CH], BF16, name="wv1")
        for wt, w_r, c, second in (
            (wq0, wq_r, 0, nc.gpsimd),
            (wq1, wq_r, 1, nc.gpsimd),
            (wv0, wv_r, 0, nc.scalar),
            (wv1, wv_r, 1, nc.scalar),
        ):
            nc.sync.dma_start(wt[:, 0:4, :], w_r[:, 0:4, c * CH : (c + 1) * CH])
            second.dma_start(wt[:, 4:8, :], w_r[:, 4:8, c * CH : (c + 1) * CH])
        # bv broadcast across partitions (V evictions only, late; its issue
        # costs ~4.6us of scalar-queue time, after everything that matters)
        bv_b = consts.tile([P, D], F32)
        nc.scalar.dma_start(bv_b, bv[None, :].broadcast_to([P, D]))
        # causal mask fill (consumed at the scores phase)
        nc.gpsimd.affine_select(
            out=cmask,
            in_=cmask,
            compare_op=mybir.AluOpType.is_ge,
            fill=MASK_VAL,
            base=0,
            pattern=[[-1, P]],
            channel_multiplier=1,
        )

        def project_chunk(ring, wt, c, dst, bias_t):
            # dst[d_part, do, s] (+bias per-partition) for d-chunk c;
            # evictions alternate Act/DVE
            for dj in range(CH // P):
                do = c * (CH // P) + dj
                for ch in range(S // CH):
                    ps = ring.tile([P, CH], F32, tag="acc", name="ps")
                    for eo in range(EO):
                        nc.tensor.matmul(
                            ps,
                            lhsT=wt[:, eo, dj * P : (dj + 1) * P],
                            rhs=x_sb[:, eo, ch * CH : (ch + 1) * CH],
                            start=(eo == 0),
                            stop=(eo == EO - 1),
                        )
                    if (dj + ch) % 2 == 0:
                        nc.scalar.activation(
                            dst[:, do, ch * CH : (ch + 1) * CH],
                            ps,
                            mybir.ActivationFunctionType.Identity,
                            bias=bias_t[:, do : do + 1],
                            scale=1.0,
                        )
                    else:
                        nc.vector.tensor_scalar(
                            dst[:, do, ch * CH : (ch + 1) * CH],
                            ps,
                            bias_t[:, do : do + 1],
                            None,
                            mybir.AluOpType.add,
                        )

        lsums = {}
        rinvs = {}

        # ---- ring 1: boot (kT c0) + kT c1 + qT + scores + V blocks 0/1,
        # all through one 8-slot [128,512] ring: bank reuse is FIFO in
        # eviction order ----
        with tc.tile_pool(name="ring1", bufs=8, space="PSUM") as ring1:
            # ch-major: the first 4 matmuls of each eo round share one
            # 512-col x block, so the boot starts on the earliest arrival
            groups = [(dj, ch) for ch in range(S // CH) for dj in range(CH // P)]
            boot_tiles = [
                ring1.tile([P, CH], F32, tag="acc", name=f"bps_{g}")
                for g in range(len(groups))
            ]
            # eo-outer over all 8 banks: consumption (~3.4us/eo cold)
            # tracks the early x arrival rate
            for eo in range(EO):
                for g, (dj, ch) in enumerate(groups):
                    nc.tensor.matmul(
                        boot_tiles[g],
                        lhsT=wk0[:, eo, dj * P : (dj + 1) * P],
                        rhs=x_sb[:, eo, ch * CH : (ch + 1) * CH],
                        start=(eo == 0),
                        stop=(eo == EO - 1),
                    )
            for g, (dj, ch) in enumerate(groups):
                if g % 2 == 0:
                    nc.scalar.activation(
                        kT_sb[:, dj, ch * CH : (ch + 1) * CH],
                        boot_tiles[g],
                        mybir.ActivationFunctionType.Identity,
                        bias=bk_t[:, dj : dj + 1],
                        scale=1.0,
                    )
                else:
                    nc.vector.tensor_scalar(
                        kT_sb[:, dj, ch * CH : (ch + 1) * CH],
                        boot_tiles[g],
                        bk_t[:, dj : dj + 1],
                        None,
                        mybir.AluOpType.add,
                    )

            project_chunk(ring1, wk1, 1, kT_sb, bk_t)
            project_chunk(ring1, wq0, 0, qT_sb, bq_t)
            project_chunk(ring1, wq1, 1, qT_sb, bq_t)

            # ---- scores, descending i, in 512-col chunks on the same ring.
            # Per chunk: 8 accumulating matmuls over do, diag mask (DVE) on
            # the last chunk, exp on Act with fused per-chunk row-sum.
            for i in reversed(range(SO)):
                kw = (i + 1) * P
                nch = (kw + CH - 1) // CH
                for ch in range(nch):
                    w = min(CH, kw - ch * CH)
                    ps_c = ring1.tile([P, CH], F32, tag="acc", name="ps_sc")
                    for do in range(DO):
                        nc.tensor.matmul(
                            ps_c[:, 0:w],
                            lhsT=qT_sb[:, do, i * P : (i + 1) * P],
                            rhs=kT_sb[:, do, ch * CH : ch * CH + w],
                            start=(do == 0),
                            stop=(do == DO - 1),
                        )
                    if ch == nch - 1:
                        # additive causal mask on the diagonal block
                        nc.vector.tensor_tensor(
                            ps_c[:, w - P : w],
                            ps_c[:, w - P : w],
                            cmask,
                            mybir.AluOpType.add,
                        )
                    lsum = small.tile([P, 1], F32, tag="lsum", name=f"ls{i}_{ch}")
                    nc.scalar.activation(
                        p_all[:, i, ch * CH : ch * CH + w],
                        ps_c[:, 0:w],
                        mybir.ActivationFunctionType.Exp,
                        bias=0.0,
                        scale=1.0,
                        accum_out=lsum,
                    )
                    lsums[(i, ch)] = lsum

            # V c0 blocks 0/1 ride the tail of ring1 (slots freed many
            # groups ago) so the ring1->ring2 handoff hides under them
            for so in (0, 1):
                ps = ring1.tile([P, CH], F32, tag="acc", name="ps_v")
                for eo in range(EO):
                    nc.tensor.matmul(
                        ps,
                        lhsT=x_sb[:, eo, so * P : (so + 1) * P],
                        rhs=wv0[:, eo, :],
                        start=(eo == 0),
                        stop=(eo == EO - 1),
                    )
                nc.vector.tensor_tensor(
                    v_sb[:, so, 0:CH],
                    ps,
                    bv_b[:, 0:CH],
                    mybir.AluOpType.add,
                )

        # lsum merges + reciprocals (DVE, tiny): DVE is strict FIFO and
        # these are exp-gated, so interleaving them with masks would
        # head-of-line-block the scores bank rotation. Consumed only by
        # the AV evictions much later.
        for i in range(SO):
            nch = (i + 1 + 3) // 4
            rinv = small.tile([P, 1], F32, tag="rinv", name=f"rinv{i}")
            if nch == 1:
                nc.vector.reciprocal(rinv, lsums[(i, 0)])
            else:
                tot = small.tile([P, 1], F32, tag="ltot", name=f"lt{i}")
                nc.vector.tensor_tensor(
                    tot, lsums[(i, 0)], lsums[(i, 1)], mybir.AluOpType.add
                )
                nc.vector.reciprocal(rinv, tot)
            rinvs[i] = rinv

        # ---- ring 2 (rest of V + AV accumulations) + transpose staging ----
        ring2 = ctx.enter_context(tc.tile_pool(name="ring2", bufs=6, space="PSUM"))
        tr_ps = ctx.enter_context(tc.tile_pool(name="tr_ps", bufs=2, space="PSUM"))
        pt_pool = ctx.enter_context(tc.tile_pool(name="pt_pool", bufs=5))
        out_pool = ctx.enter_context(tc.tile_pool(name="out_pool", bufs=4))

        def emit_transposes(i):
            # pT_i = p_i blocks transposed via the PE; copies drain on DVE
            pT = pt_pool.tile([P, S], BF16, tag="pT", name=f"pT{i}")
            for j in range(i + 1):
                ps_t = tr_ps.tile([P, P], BF16, tag="tr", name="ps_t")
                nc.tensor.transpose(
                    ps_t, p_all[:, i, j * P : (j + 1) * P], identity
                )
                nc.vector.tensor_copy(pT[:, j * P : (j + 1) * P], ps_t)
            return pT

        def emit_av(i):
            # out_i = (pT_i.T @ v) * rinv_i, evicted per 512-chunk; out DMAs
            # rotate over three rings so writeback streams during the phase.
            nk = i + 1
            rinv = rinvs[i]
            pT = pts.pop(i)
            out_sb = out_pool.tile([P, D], BF16, tag="out", name="out_sb")
            last = i == SO - 1
            rings = (nc.gpsimd, nc.sync, nc.scalar)
            for c2 in range(NCH):
                ps_o = ring2.tile([P, CH], F32, tag="av", name="ps_o")
                for j in range(nk):
                    nc.tensor.matmul(
                        ps_o,
                        lhsT=pT[:, j * P : (j + 1) * P],
                        rhs=v_sb[:, j, c2 * CH : (c2 + 1) * CH],
                        start=(j == 0),
                        stop=(j == nk - 1),
                    )
                base = c2 * CH
                if last and c2 == NCH - 1:
                    # final tile: eviction split DVE+Act, output quartered
                    # across the rings to shorten the tail drain
                    half = CH // 2
                    qt = CH // 4
                    nc.vector.tensor_scalar_mul(
                        out_sb[:, base : base + half], ps_o[:, 0:half], rinv
                    )
                    nc.scalar.activation(
                        out_sb[:, base + half : base + CH],
                        ps_o[:, half:CH],
                        mybir.ActivationFunctionType.Identity,
                        bias=0.0,
                        scale=rinv,
                    )
                    for qi, ring in enumerate(
                        (nc.gpsimd, nc.sync, nc.scalar, nc.sync)
                    ):
                        ring.dma_start(
                            out[
                                i * P : (i + 1) * P,
                                base + qi * qt : base + (qi + 1) * qt,
                            ],
                            out_sb[:, base + qi * qt : base + (qi + 1) * qt],
                        )
                else:
                    # alternate evictions DVE/Act so neither queue stalls
                    # the next tile's pT copies
                    if c2 % 2 == 0:
                        nc.vector.tensor_scalar_mul(
                            out_sb[:, base : base + CH], ps_o, rinv
                        )
                    else:
                        nc.scalar.activation(
                            out_sb[:, base : base + CH],
                            ps_o,
                            mybir.ActivationFunctionType.Identity,
                            bias=0.0,
                            scale=rinv,
                        )
                    ring = rings[(i * NCH + c2) % 3]
                    ring.dma_start(
                        out[i * P : (i + 1) * P, base : base + CH],
                        out_sb[:, base : base + CH],
                    )

        # ---- V chunk 0 (blocks 2-7), then V chunk 1 fused with AV ----
        pts = {}
        for so in range(2, SO):
            ps = ring2.tile([P, CH], F32, tag="av", name="ps_v")
            for eo in range(EO):
                nc.tensor.matmul(
                    ps,
                    lhsT=x_sb[:, eo, so * P : (so + 1) * P],
                    rhs=wv0[:, eo, :],
                    start=(eo == 0),
                    stop=(eo == EO - 1),
                )
            nc.vector.tensor_tensor(
                v_sb[:, so, 0:CH],
                ps,
                bv_b[:, 0:CH],
                mybir.AluOpType.add,
            )
            if so >= SO - 4:
                # pT_0..pT_3 prefire under the tail of the V c0 matmuls
                pts[so - (SO - 4)] = emit_transposes(so - (SO - 4))
        for so in range(SO):
            ps = ring2.tile([P, CH], F32, tag="av", name="ps_v")
            for eo in range(EO):
                nc.tensor.matmul(
                    ps,
                    lhsT=x_sb[:, eo, so * P : (so + 1) * P],
                    rhs=wv1[:, eo, :],
                    start=(eo == 0),
                    stop=(eo == EO - 1),
                )
            nc.vector.tensor_tensor(
                v_sb[:, so, CH : 2 * CH],
                ps,
                bv_b[:, CH : 2 * CH],
                mybir.AluOpType.add,
            )
            if so + 4 < SO:
                pts[so + 4] = emit_transposes(so + 4)
            emit_av(so)

    nc.compile()
    return nc


_NC_CACHE = None


def get_program():
    global _NC_CACHE
    if _NC_CACHE is None:
        _NC_CACHE = build_program()
    return _NC_CACHE


def make_in_maps(x, Wq, bq, Wk, bk, Wv, bv):
    x = np.asarray(x, dtype=np.float32)
    wqT = np.ascontiguousarray(
        (np.asarray(Wq, dtype=np.float32).T * np.float32(SCALE)).astype(
            ml_dtypes.bfloat16
        )
    )
    wkT = np.ascontiguousarray(np.asarray(Wk, dtype=np.float32).T.astype(ml_dtypes.bfloat16))
    wvT = np.ascontiguousarray(np.asarray(Wv, dtype=np.float32).T.astype(ml_dtypes.bfloat16))
    # biases packed to [P, DO]: element (p, o) = b[o*P + p]
    bqs = np.ascontiguousarray(
        (np.asarray(bq, dtype=np.float32) * np.float32(SCALE)).reshape(DO, P).T
    )
    bk = np.ascontiguousarray(np.asarray(bk, dtype=np.float32).reshape(DO, P).T)
    bv = np.asarray(bv, dtype=np.float32)
    in_maps = []
    for b in range(B):
        in_maps.append(
            {
                "xT": np.ascontiguousarray(x[b].T.astype(ml_dtypes.bfloat16)),
                "wqT": wqT,
                "wkT": wkT,
                "wvT": wvT,
                "bqs": bqs,
                "bk": bk,
                "bv": bv,
            }
        )
    return in_maps


def run_on_hw(in_maps, trace=False, **kwargs):
    nc = get_program()
    return run_bass_kernel_spmd(
        nc, in_maps, core_ids=list(range(B)), trace=trace, **kwargs
    )


def kernel(x, Wq, bq, Wk, bk, Wv, bv):
    in_maps = make_in_maps(x, Wq, bq, Wk, bk, Wv, bv)
    res = run_on_hw(in_maps)
    return np.stack(
        [res.results[b]["out"].astype(np.float32) for b in range(B)], axis=0
    )
